# revision 26
# baseline (speedup 1.0000x reference)
"""Trainium2 Bass kernel for local-window Bahdanau attention.

Problem (hardcoded shapes): B=1024, L=100 (10x10 grid), C=1024, U=512,
window 3x3 (D=1).  Reference computes:
    p_t   = sigmoid(tanh(h @ Wa) @ Wb) * 8 + 1          (B,1,2)
    st    = int32(p_t - 1) clamped to [0, 7]            (B,2)
    local = grid[b, st0:st0+3, st1:st1+3, :]            (B,9,C)
    score = tanh(local @ W1 + W1_b + h @ W2 + W2_b)     (B,9,U)
    attn  = softmax(score @ V1 + V1_b, axis=1) * gauss  (B,9,1)
    ctx   = sum(attn * local, axis=1)                   (B,C)
    out   = tanh(concat([ctx, h])) @ W3 + W3_b          (B,U)
returns (out, attn).

Strategy: pure data-parallel over 8 NeuronCores (128 examples each).
The 3x3 window rows are fetched with dma_gather (device-side int16
indices computed from hidden), so only 9/100 of `features` is read.
Matmul-heavy parts run in bf16 (fp32 PSUM accumulation); the index
computation path (p_t) is kept entirely in fp32 to match the
reference's truncation.
"""

import sys

if "/opt/trn_rl_repo" not in sys.path:
    sys.path.insert(0, "/opt/trn_rl_repo")

import numpy as np
import ml_dtypes

import concourse.bass as bass
import concourse.bacc as bacc
import concourse.mybir as mybir
from concourse.ap import AP
from concourse.tile import TileContext, add_dep_helper
from concourse.bass_utils import run_bass_kernel_spmd
from concourse.library_config import mlp

F32 = mybir.dt.float32
BF16 = mybir.dt.bfloat16
I16 = mybir.dt.int16
I32 = mybir.dt.int32
AF = mybir.ActivationFunctionType
ALU = mybir.AluOpType

B, L, C, U = 1024, 100, 1024, 512
G, WIN, D = 10, 3, 1
NL = WIN * WIN            # 9 window positions
NC_CORES = 8
BS = B // NC_CORES        # 128 examples per core
NROWS = BS * L            # 12800 feature rows per core
KC = C // 128             # 8 contraction chunks over C
KU = U // 128             # 4 chunks over U
KT = (C + U) // 128       # 12 chunks over C+U
RPW = NL * BS             # 1152 score rows per core

# packed bf16 weights layout (column offsets in the [128, .] tile)
W_W1, W_W2, W_W3 = 0, KC * U, (KC + KU) * U
W_EYE = (KC + KU + KT) * U
W_V1 = W_EYE + 128
W_COLS = W_V1 + KU
W2_OFF = 0          # w2cat: [w2]
# critical f32 consts (needed for the p_t/index chain)
F_ID, F_WA = 0, 128
F_MASK = F_WA + KU * 100
F_COMB = F_MASK + 128
FC_COLS = F_COMB + 128
# late f32 consts
F_W3B = 0
F_GAUSS = F_W3B + U
F_IOTA = F_GAUSS + NL
F_B12 = F_IOTA + 8
FR_COLS = F_B12 + KU


def _build_nc():
    nc = bacc.Bacc("TRN2", target_bir_lowering=False)

    feat = nc.dram_tensor("feat", [NROWS, C], F32, kind="ExternalInput")
    hid = nc.dram_tensor("hid", [BS, U], F32, kind="ExternalInput")
    fcrit = nc.dram_tensor("fcrit", [128, FC_COLS], F32, kind="ExternalInput")
    w2cat = nc.dram_tensor("w2cat", [128, KU * U], BF16, kind="ExternalInput")
    w1cat = nc.dram_tensor("w1cat", [128, KC * U], BF16, kind="ExternalInput")
    frest = nc.dram_tensor("frest", [128, FR_COLS], F32, kind="ExternalInput")
    w3cat = nc.dram_tensor("w3cat", [128, KT * U + 128 + KU], BF16,
                           kind="ExternalInput")
    wbd = nc.dram_tensor("wbd", [100, 2], F32, kind="ExternalInput")
    sgi = nc.dram_tensor("sgi", [128, 24], I16, kind="ExternalInput")

    out = nc.dram_tensor("out", [BS, U], F32, kind="ExternalOutput")
    attn = nc.dram_tensor("attn", [BS, NL], F32, kind="ExternalOutput")

    with TileContext(nc) as tc:
        lib_inst = nc.gpsimd.load_library(mlp)

        with tc.tile_pool(name="sb", bufs=1) as sb:
            # ---- persistent SBUF tiles ----
            h_nat = sb.tile([BS, U], F32, tag="h_nat")
            hT32 = sb.tile([128, U], F32, tag="hT32")
            hT16 = sb.tile([128, U], BF16, tag="hT16")
            th16 = sb.tile([128, U], BF16, tag="th16")
            w_all = sb.tile([128, W_COLS], BF16, tag="w_all")
            fc_all = sb.tile([128, FC_COLS], F32, tag="fc_all")
            fr_all = sb.tile([128, FR_COLS], F32, tag="fr_all")
            wb_sb = sb.tile([100, 2], F32, tag="wb_sb")

            w1_sb = w_all[:, W_W1:W_W1 + KC * U]
            w2_sb = w_all[:, W_W2:W_W2 + KU * U]
            w3_sb = w_all[:, W_W3:W_W3 + KT * U]
            eye_sb = w_all[:, W_EYE:W_EYE + 128]
            v1_sb = w_all[:, W_V1:W_V1 + KU]
            id_sb = fc_all[:, F_ID:F_ID + 128]
            wa_sb = fc_all[:, F_WA:F_WA + KU * 100]
            mask_sb = fc_all[:, F_MASK:F_MASK + 128]
            comb2_sb = fc_all[0:2, F_COMB:F_COMB + 128]
            w3b_sb = fr_all[:, F_W3B:F_W3B + U]
            gauss_sb = fr_all[:, F_GAUSS:F_GAUSS + NL]
            iota_sb = fr_all[:, F_IOTA:F_IOTA + 8]
            b12_sb = fr_all[:, F_B12:F_B12 + KU]

            local_nat = sb.tile([128, NL * C], F32, tag="local_nat")
            local_bf = sb.tile([128, NL * C], BF16, tag="local_bf")
            localT0 = sb.tile([128, KC * 384], BF16, tag="localT0")
            localT1 = sb.tile([128, KC * 384], BF16, tag="localT1")
            localT2 = sb.tile([128, KC * 384], BF16, tag="localT2")
            sgidx = sb.tile([128, 24], I16, tag="sgidx")
            w2h_sb = sb.tile([128, KU * 128], F32, tag="w2h_sb")
            scoreT = sb.tile([128, KU * RPW], BF16, tag="scoreT")
            tct16 = sb.tile([128, KC * 128], BF16, tag="tct16")
            diag = sb.tile([128, NL * 128], BF16, tag="diag")
            out_sb = sb.tile([BS, U], F32, tag="out_sb")

            t1 = sb.tile([128, 128], F32, tag="t1")
            t2 = sb.tile([2, 128], F32, tag="t2")
            pm1 = sb.tile([2, 128], F32, tag="pm1")
            ci = sb.tile([2, 128], I32, tag="ci")
            cf = sb.tile([2, 128], F32, tag="cf")
            gt = sb.tile([2, 128], F32, tag="gt")
            st = sb.tile([2, 128], F32, tag="st")
            s16m = sb.tile([128, 128], F32, tag="s16m")
            base16 = sb.tile([128, 8], F32, tag="base16")
            idxf = sb.tile([128, 24], F32, tag="idxf")
            idx16 = sb.tile([128, 24], I16, tag="idx16")
            stmp = sb.tile([128, RPW], F32, tag="stmp")
            negmax = sb.tile([128, 1], F32, tag="negmax")
            esum = sb.tile([128, 1], F32, tag="esum")
            rsum = sb.tile([128, 1], F32, tag="rsum")
            e_sb = sb.tile([128, NL], F32, tag="e_sb")
            attn_f = sb.tile([128, NL], F32, tag="attn_f")

            # ---- input DMAs: one FIFO ring (Sync), priority order ----
            nc.sync.dma_start(h_nat[:], hid[:])
            nc.sync.dma_start(fc_all[:], fcrit[:])
            nc.sync.dma_start(wb_sb[:], wbd[:])
            nc.sync.dma_start(sgidx[:], sgi[:])
            nc.sync.dma_start(w_all[:, W_W2:W_W2 + KU * U], w2cat[:])
            nc.sync.dma_start(w_all[:, W_W1:W_W1 + KC * U], w1cat[:])
            nc.sync.dma_start(fr_all[:], frest[:])
            nc.sync.dma_start(w_all[:, W_W3:], w3cat[:])

            # ---- hT (PE transpose, fp32) ----
            with tc.tile_pool(name="pmps", bufs=2, space="PSUM") as pmps:
                for k in range(KU):
                    ps = pmps.tile([128, 128], F32, tag="ps_tr", name=f"ptr{k}")
                    nc.tensor.transpose(ps[:], h_nat[:, k * 128:(k + 1) * 128], id_sb)
                    nc.vector.tensor_copy(hT32[:, k * 128:(k + 1) * 128], ps[:])
                # ---- p_t chain (fp32 end to end) ----
                z1 = pmps.tile([128, 128], F32, tag="ps_z")
                for k in range(KU):
                    nc.tensor.matmul(z1[0:100, :], wa_sb[:, k * 100:(k + 1) * 100],
                                     hT32[:, k * 128:(k + 1) * 128],
                                     start=(k == 0), stop=(k == KU - 1))
                nc.scalar.activation(t1[0:100, :], z1[0:100, :], AF.Tanh)
                z2 = pmps.tile([128, 128], F32, tag="ps_z")
                nc.tensor.matmul(z2[0:2, :], wb_sb[:], t1[0:100, :], start=True, stop=True)
                # p_t - 1 = 8*sigmoid(z) = 4*tanh(z/2) + 4
                nc.scalar.activation(t2[:], z2[0:2, :], AF.Tanh, scale=0.5)
                nc.vector.tensor_scalar(pm1[:], t2[:], 4.0, 4.0, ALU.mult, ALU.add)
                # floor (rounding-mode agnostic): c=int(x); c -= (c > x)
                nc.vector.tensor_copy(ci[:], pm1[:])
                nc.vector.tensor_copy(cf[:], ci[:])
                nc.vector.tensor_tensor(gt[:], cf[:], pm1[:], ALU.is_gt)
                nc.vector.tensor_tensor(st[:], cf[:], gt[:], ALU.subtract)
                # (clamp omitted: p_t-1 in (0,8) strictly, so floor in [0,7])
                # s16[m, b] = 10*st0[b] + st1[b] on all 128 partitions
                s16 = pmps.tile([128, 128], F32, tag="ps_z")
                nc.tensor.matmul(s16[:], comb2_sb, st[:], start=True, stop=True)
                # diagonal extract: base16[p, q] = s16[p, q*16 + p%16]
                nc.vector.tensor_tensor(s16m[:], s16[:], mask_sb, ALU.mult)
                nc.vector.reduce_sum(base16[:],
                                     s16m[:].rearrange("p (q i) -> p q i", i=16),
                                     axis=mybir.AxisListType.X)
                for j in range(WIN):
                    nc.vector.scalar_tensor_tensor(
                        idx16[:, j * 8:(j + 1) * 8], base16[:], float(G * j),
                        iota_sb, ALU.add, ALU.add)

            # ---- gather the 3x3 windows (3 gathers: one grid-row each) ----
            feat_gap = AP(feat.ap().tensor, 0, [[C, NROWS - 2], [1, WIN * C]])
            gathers = []
            for j in range(WIN):
                g = nc.gpsimd.dma_gather(
                    local_nat[:, j * WIN * C:(j + 1) * WIN * C]
                        .rearrange("p (o e) -> p o e", o=1),
                    feat_gap,
                    idx16[:, j * 8:(j + 1) * 8],
                    BS, BS, WIN * C,
                    elem_step=C,
                )
                add_dep_helper(g.ins, lib_inst.ins, reason="load_library before gather")
                gathers.append(g)

            with tc.tile_pool(name="pm2", bufs=2, space="PSUM") as pm2:
                # ---- w2h = (h @ W2)^T  [uo*128+m, b], one psum bank ----
                nc.vector.tensor_copy(hT16[:], hT32[:])
                psw = pm2.tile([128, 512], F32, tag="ps_w2h")
                for uo in range(KU):
                    for ui in range(KU):
                        nc.tensor.matmul(
                            psw[:, uo * 128:(uo + 1) * 128],
                            w2_sb[:, ui * U + uo * 128: ui * U + (uo + 1) * 128],
                            hT16[:, ui * 128:(ui + 1) * 128],
                            start=(ui == 0), stop=(ui == KU - 1))
                nc.vector.tensor_copy(w2h_sb[:], psw[:])
                # tanh(h)^T for the W3 matmul tail
                nc.scalar.activation(th16[:], hT32[:], AF.Tanh)

                # PE warmup during the gather/transpose window (keeps HAM at
                # full clock); results are never read.
                warm = pm2.tile([128, 512], F32, tag="ps_warm")
                for i in range(56):
                    nc.tensor.matmul(warm[:], w1_sb[:, 0:128],
                                     w1_sb[:, (i % 7) * 512:(i % 7) * 512 + 512],
                                     start=True, stop=True)

            # cast to bf16, then SBUF-source transpose gathers (SWDGE):
            # localT_j[c0, cc*384 + i] = local_bf[b, l*C + cc*128 + c0],
            # i = (l - 3j)*128 + b
            localTs = [localT0, localT1, localT2]
            for j in range(WIN):
                nc.vector.tensor_copy(local_bf[:, j * WIN * C:(j + 1) * WIN * C],
                                      local_nat[:, j * WIN * C:(j + 1) * WIN * C])
                sg = nc.gpsimd.dma_gather(
                    localTs[j][:].rearrange("p (k i) -> p k i", k=KC),
                    local_bf[:, j * WIN * C:(j + 1) * WIN * C],
                    sgidx[:],
                    384, 384, C,
                    transpose=True,
                    sbuf_tokens_per_rank=128,
                    sbuf_free_dim_per_rank=2 * C,
                )
                add_dep_helper(sg.ins, lib_inst.ins, reason="lib before sg")

            # ---- scoreT = tanh(W1^T localT + w2h + b12)  [u, l*128+b] ----
            with tc.tile_pool(name="sps", bufs=2, space="PSUM") as sps:
                for uo in range(KU):
                    pss = [sps.tile([128, 384], F32, tag=f"ps_s{j}",
                                    name=f"pss{uo}_{j}")
                           for j in range(WIN)]
                    for k in range(KC):
                        for j in range(WIN):
                            nc.tensor.matmul(
                                pss[j][:],
                                w1_sb[:, k * U + uo * 128:k * U + (uo + 1) * 128],
                                localTs[j][:, k * 384:(k + 1) * 384],
                                start=(k == 0), stop=(k == KC - 1))
                    for j in range(WIN):
                        nc.vector.tensor_tensor(
                            stmp[:, j * 384:(j + 1) * 384]
                                .rearrange("p (l b) -> p l b", b=128),
                            pss[j][:].rearrange("p (l b) -> p l b", b=128),
                            w2h_sb[:, uo * 128:(uo + 1) * 128].unsqueeze(1)
                                .broadcast_to([128, WIN, 128]),
                            ALU.add)
                    nc.scalar.activation(scoreT[:, uo * RPW:(uo + 1) * RPW], stmp[:],
                                         AF.Tanh, bias=b12_sb[:, uo:uo + 1])

            # ---- logits -> softmax -> attn ----
            with tc.tile_pool(name="lgps", bufs=1, space="PSUM") as lgps:
                lg = lgps.tile([128, NL], F32, tag="ps_lg")
                for l in range(NL):
                    for uo in range(KU):
                        nc.tensor.matmul(
                            lg[:, l:l + 1],
                            scoreT[:, uo * RPW + l * 128:uo * RPW + (l + 1) * 128],
                            v1_sb[:, uo:uo + 1],
                            start=(uo == 0), stop=(uo == KU - 1))
                nc.vector.tensor_reduce(negmax[:], lg[:], axis=mybir.AxisListType.X,
                                        op=ALU.max, negate=True)
                nc.scalar.activation(e_sb[:], lg[:], AF.Exp, bias=negmax[:])
            nc.vector.reduce_sum(esum[:], e_sb[:], axis=mybir.AxisListType.X)
            nc.vector.reciprocal(rsum[:], esum[:])
            nc.vector.scalar_tensor_tensor(attn_f[:], e_sb[:], rsum[:], gauss_sb,
                                           ALU.mult, ALU.mult)
            nc.sync.dma_start(attn[:], attn_f[:])

            # ---- ctx^T via diag matmuls, then tanh -> tct16 ----
            for l in range(NL):
                nc.vector.tensor_scalar_mul(diag[:, l * 128:(l + 1) * 128], eye_sb,
                                            attn_f[:, l:l + 1])
            with tc.tile_pool(name="cps", bufs=4, space="PSUM") as cps, \
                 tc.tile_pool(name="ops", bufs=1, space="PSUM") as ops:
                for cc in range(KC):
                    pc = cps.tile([128, 128], F32, tag="ps_c", name=f"pc{cc}")
                    for l in range(NL):
                        nc.tensor.matmul(pc[:],
                                         local_bf[:, l * C + cc * 128:l * C + (cc + 1) * 128],
                                         diag[:, l * 128:(l + 1) * 128],
                                         start=(l == 0), stop=(l == NL - 1))
                    nc.scalar.activation(tct16[:, cc * 128:(cc + 1) * 128], pc[:], AF.Tanh)

                # ---- out = tanh([ctx, h]) @ W3 + W3_b ----
                po = ops.tile([128, U], F32, tag="ps_o")
                for kk in range(KT):
                    lhsT = (tct16[:, kk * 128:(kk + 1) * 128] if kk < KC
                            else th16[:, (kk - KC) * 128:(kk - KC + 1) * 128])
                    nc.tensor.matmul(po[:], lhsT, w3_sb[:, kk * U:(kk + 1) * U],
                                     start=(kk == 0), stop=(kk == KT - 1))
                nc.vector.tensor_tensor(out_sb[:], po[:], w3b_sb, ALU.add)
            nc.sync.dma_start(out[:], out_sb[:])

    nc.compile()
    return nc


_NC_CACHE = None


def _get_nc():
    global _NC_CACHE
    if _NC_CACHE is None:
        _NC_CACHE = _build_nc()
    return _NC_CACHE


def _chunked(w, k):
    """[k*128, n] -> [128, k*n] with chunk-major columns."""
    n = w.shape[1]
    return np.ascontiguousarray(
        w.reshape(k, 128, n).transpose(1, 0, 2).reshape(128, k * n))


def make_host_inputs(features, hidden, W1_w, W1_b, W2_w, W2_b, V1_w, V1_b,
                     W3_w, W3_b, Wa, Wb):
    """Build the 8 per-core input maps."""
    bf = ml_dtypes.bfloat16
    f = np.float32

    jj, kk = np.meshgrid(np.arange(WIN), np.arange(WIN), indexing="ij")
    d2 = (jj - WIN / 2.0) ** 2 + (kk - WIN / 2.0) ** 2
    gauss_row = np.exp(-d2 / (0.5 * D * D)).reshape(NL).astype(f)

    p = np.arange(128)
    q = np.arange(8)

    w3cat = np.zeros((128, KT * U + 128 + KU), bf)
    w3cat[:, 0:KT * U] = _chunked(np.asarray(W3_w, f), KT).astype(bf)
    w3cat[:, KT * U:KT * U + 128] = np.eye(128, dtype=f).astype(bf)
    w3cat[:, KT * U + 128:] = _chunked(np.asarray(V1_w, f), KU).astype(bf)

    fcrit = np.zeros((128, FC_COLS), f)
    fcrit[:, F_ID:F_ID + 128] = np.eye(128, dtype=f)
    fcrit[:, F_WA:F_WA + KU * 100] = _chunked(np.asarray(Wa, f), KU)
    fcrit[:, F_MASK:F_MASK + 128] = (
        np.arange(128)[None, :] % 16 == p[:, None] % 16)
    fcrit[0, F_COMB:F_COMB + 128] = float(G)
    fcrit[1, F_COMB:F_COMB + 128] = 1.0

    frest = np.zeros((128, FR_COLS), f)
    frest[:, F_W3B:F_GAUSS] = np.broadcast_to(np.asarray(W3_b, f), (128, U))
    frest[:, F_GAUSS:F_IOTA] = np.broadcast_to(gauss_row, (128, NL))
    frest[:, F_IOTA:F_B12] = L * (q[None, :] * 16 + (p[:, None] % 16))
    frest[:, F_B12:F_B12 + KU] = _chunked(
        (np.asarray(W1_b, f) + np.asarray(W2_b, f)).reshape(U, 1), KU)

    si = np.arange(24)[None, :] * 16 + (np.arange(128)[:, None] % 16)
    shared = {
        "sgi": si.astype(np.int16),
        "fcrit": fcrit,
        "frest": frest,
        "w1cat": _chunked(np.asarray(W1_w, f), KC).astype(bf),
        "w2cat": _chunked(np.asarray(W2_w, f), KU).astype(bf),
        "w3cat": w3cat,
        "wbd": np.ascontiguousarray(Wb, f),
    }
    features = np.asarray(features, f)
    hidden = np.asarray(hidden, f)
    in_maps = []
    for c in range(NC_CORES):
        sl = slice(c * BS, (c + 1) * BS)
        m = dict(shared)
        m["feat"] = np.ascontiguousarray(features[sl]).reshape(NROWS, C)
        m["hid"] = np.ascontiguousarray(hidden[sl])
        in_maps.append(m)
    return in_maps


def kernel(features, hidden, W1_w, W1_b, W2_w, W2_b, V1_w, V1_b,
           W3_w, W3_b, Wa, Wb, _run_kwargs=None):
    nc = _get_nc()
    in_maps = make_host_inputs(features, hidden, W1_w, W1_b, W2_w, W2_b,
                               V1_w, V1_b, W3_w, W3_b, Wa, Wb)
    res = run_bass_kernel_spmd(nc, in_maps, core_ids=list(range(NC_CORES)),
                               **(_run_kwargs or {}))
    out = np.concatenate([r["out"] for r in res.results], axis=0)
    attn = np.concatenate([r["attn"] for r in res.results], axis=0)
    kernel.last_results = res
    return out, attn.reshape(B, NL, 1)


# revision 28
# speedup vs baseline: 1.0103x; 1.0103x over previous
"""Trainium2 Bass kernel for local-window Bahdanau attention.

Problem (hardcoded shapes): B=1024, L=100 (10x10 grid), C=1024, U=512,
window 3x3 (D=1).  Reference computes:
    p_t   = sigmoid(tanh(h @ Wa) @ Wb) * 8 + 1          (B,1,2)
    st    = int32(p_t - 1) clamped to [0, 7]            (B,2)
    local = grid[b, st0:st0+3, st1:st1+3, :]            (B,9,C)
    score = tanh(local @ W1 + W1_b + h @ W2 + W2_b)     (B,9,U)
    attn  = softmax(score @ V1 + V1_b, axis=1) * gauss  (B,9,1)
    ctx   = sum(attn * local, axis=1)                   (B,C)
    out   = tanh(concat([ctx, h])) @ W3 + W3_b          (B,U)
returns (out, attn).

Strategy: pure data-parallel over 8 NeuronCores (128 examples each).
The 3x3 window rows are fetched with dma_gather (device-side int16
indices computed from hidden), so only 9/100 of `features` is read.
Matmul-heavy parts run in bf16 (fp32 PSUM accumulation); the index
computation path (p_t) is kept entirely in fp32 to match the
reference's truncation.
"""

import sys

if "/opt/trn_rl_repo" not in sys.path:
    sys.path.insert(0, "/opt/trn_rl_repo")

import numpy as np
import ml_dtypes

import concourse.bass as bass
import concourse.bacc as bacc
import concourse.mybir as mybir
from concourse.ap import AP
from concourse.tile import TileContext, add_dep_helper
from concourse.bass_utils import run_bass_kernel_spmd
from concourse.library_config import mlp

F32 = mybir.dt.float32
BF16 = mybir.dt.bfloat16
I16 = mybir.dt.int16
I32 = mybir.dt.int32
AF = mybir.ActivationFunctionType
ALU = mybir.AluOpType

B, L, C, U = 1024, 100, 1024, 512
G, WIN, D = 10, 3, 1
NL = WIN * WIN            # 9 window positions
NC_CORES = 8
BS = B // NC_CORES        # 128 examples per core
NROWS = BS * L            # 12800 feature rows per core
KC = C // 128             # 8 contraction chunks over C
KU = U // 128             # 4 chunks over U
KT = (C + U) // 128       # 12 chunks over C+U
RPW = NL * BS             # 1152 score rows per core

# packed bf16 weights layout (column offsets in the [128, .] tile)
W_W1, W_W2, W_W3 = 0, KC * U, (KC + KU) * U
W_EYE = (KC + KU + KT) * U
W_V1 = W_EYE + 128
W_COLS = W_V1 + KU
W2_OFF = 0          # w2cat: [w2]
# critical f32 consts (needed for the p_t/index chain)
F_ID, F_WA = 0, 128
F_MASK = F_WA + KU * 100
F_COMB = F_MASK + 128
FC_COLS = F_COMB + 128
# late f32 consts
F_W3B = 0
F_GAUSS = F_W3B + U
F_IOTA = F_GAUSS + NL
F_B12 = F_IOTA + 8
FR_COLS = F_B12 + KU


def _build_nc():
    nc = bacc.Bacc("TRN2", target_bir_lowering=False)

    feat = nc.dram_tensor("feat", [NROWS, C], F32, kind="ExternalInput")
    hid = nc.dram_tensor("hid", [BS, U], F32, kind="ExternalInput")
    fcrit = nc.dram_tensor("fcrit", [128, FC_COLS], F32, kind="ExternalInput")
    w2cat = nc.dram_tensor("w2cat", [128, KU * U], BF16, kind="ExternalInput")
    w1cat = nc.dram_tensor("w1cat", [128, KC * U], BF16, kind="ExternalInput")
    frest = nc.dram_tensor("frest", [128, FR_COLS], F32, kind="ExternalInput")
    w3cat = nc.dram_tensor("w3cat", [128, KT * U + 128 + KU], BF16,
                           kind="ExternalInput")
    wbd = nc.dram_tensor("wbd", [100, 2], F32, kind="ExternalInput")
    sgi = nc.dram_tensor("sgi", [128, 24], I16, kind="ExternalInput")

    out = nc.dram_tensor("out", [BS, U], F32, kind="ExternalOutput")
    attn = nc.dram_tensor("attn", [BS, NL], F32, kind="ExternalOutput")

    with TileContext(nc) as tc:
        lib_inst = nc.gpsimd.load_library(mlp)

        with tc.tile_pool(name="sb", bufs=1) as sb:
            # ---- persistent SBUF tiles ----
            h_nat = sb.tile([BS, U], F32, tag="h_nat")
            hT32 = sb.tile([128, U], F32, tag="hT32")
            hT16 = sb.tile([128, U], BF16, tag="hT16")
            th16 = sb.tile([128, U], BF16, tag="th16")
            w_all = sb.tile([128, W_COLS], BF16, tag="w_all")
            fc_all = sb.tile([128, FC_COLS], F32, tag="fc_all")
            fr_all = sb.tile([128, FR_COLS], F32, tag="fr_all")
            wb_sb = sb.tile([100, 2], F32, tag="wb_sb")

            w1_sb = w_all[:, W_W1:W_W1 + KC * U]
            w2_sb = w_all[:, W_W2:W_W2 + KU * U]
            w3_sb = w_all[:, W_W3:W_W3 + KT * U]
            eye_sb = w_all[:, W_EYE:W_EYE + 128]
            v1_sb = w_all[:, W_V1:W_V1 + KU]
            id_sb = fc_all[:, F_ID:F_ID + 128]
            wa_sb = fc_all[:, F_WA:F_WA + KU * 100]
            mask_sb = fc_all[:, F_MASK:F_MASK + 128]
            comb2_sb = fc_all[0:2, F_COMB:F_COMB + 128]
            w3b_sb = fr_all[:, F_W3B:F_W3B + U]
            gauss_sb = fr_all[:, F_GAUSS:F_GAUSS + NL]
            iota_sb = fr_all[:, F_IOTA:F_IOTA + 8]
            b12_sb = fr_all[:, F_B12:F_B12 + KU]

            local_nat = sb.tile([128, NL * C], F32, tag="local_nat")
            local_bf = sb.tile([128, NL * C], BF16, tag="local_bf")
            localT0 = sb.tile([128, KC * 384], BF16, tag="localT0")
            localT1 = sb.tile([128, KC * 384], BF16, tag="localT1")
            localT2 = sb.tile([128, KC * 384], BF16, tag="localT2")
            sgidx = sb.tile([128, 24], I16, tag="sgidx")
            w2h_sb = sb.tile([128, KU * 128], F32, tag="w2h_sb")
            scoreT = sb.tile([128, KU * RPW], BF16, tag="scoreT")
            tct16 = sb.tile([128, KC * 128], BF16, tag="tct16")
            diag = sb.tile([128, NL * 128], BF16, tag="diag")
            out_sb = sb.tile([BS, U], F32, tag="out_sb")

            t1 = sb.tile([128, 128], F32, tag="t1")
            t2 = sb.tile([2, 128], F32, tag="t2")
            pm1 = sb.tile([2, 128], F32, tag="pm1")
            ci = sb.tile([2, 128], I32, tag="ci")
            cf = sb.tile([2, 128], F32, tag="cf")
            gt = sb.tile([2, 128], F32, tag="gt")
            st = sb.tile([2, 128], F32, tag="st")
            s16m = sb.tile([128, 128], F32, tag="s16m")
            base16 = sb.tile([128, 8], F32, tag="base16")
            idxf = sb.tile([128, 24], F32, tag="idxf")
            idx16 = sb.tile([128, 24], I16, tag="idx16")
            stmp = sb.tile([128, RPW], F32, tag="stmp")
            negmax = sb.tile([128, 1], F32, tag="negmax")
            esum = sb.tile([128, 1], F32, tag="esum")
            rsum = sb.tile([128, 1], F32, tag="rsum")
            e_sb = sb.tile([128, NL], F32, tag="e_sb")
            attn_f = sb.tile([128, NL], F32, tag="attn_f")

            # ---- input DMAs: one FIFO ring (Sync), priority order ----
            nc.sync.dma_start(h_nat[:], hid[:])
            nc.sync.dma_start(fc_all[:], fcrit[:])
            nc.sync.dma_start(wb_sb[:], wbd[:])
            nc.sync.dma_start(sgidx[:], sgi[:])
            nc.sync.dma_start(w_all[:, W_W2:W_W2 + KU * U], w2cat[:])
            nc.sync.dma_start(w_all[:, W_W1:W_W1 + KC * U], w1cat[:])
            nc.sync.dma_start(fr_all[:], frest[:])
            nc.sync.dma_start(w_all[:, W_W3:], w3cat[:])

            # ---- hT (PE transpose, fp32) ----
            with tc.tile_pool(name="pmps", bufs=2, space="PSUM") as pmps:
                for k in range(KU):
                    ps = pmps.tile([128, 128], F32, tag="ps_tr", name=f"ptr{k}")
                    nc.tensor.transpose(ps[:], h_nat[:, k * 128:(k + 1) * 128], id_sb)
                    nc.vector.tensor_copy(hT32[:, k * 128:(k + 1) * 128], ps[:])
                # ---- p_t chain (fp32 end to end) ----
                z1 = pmps.tile([128, 128], F32, tag="ps_z")
                for k in range(KU):
                    nc.tensor.matmul(z1[0:100, :], wa_sb[:, k * 100:(k + 1) * 100],
                                     hT32[:, k * 128:(k + 1) * 128],
                                     start=(k == 0), stop=(k == KU - 1))
                nc.scalar.activation(t1[0:100, :], z1[0:100, :], AF.Tanh)
                z2 = pmps.tile([128, 128], F32, tag="ps_z")
                nc.tensor.matmul(z2[0:2, :], wb_sb[:], t1[0:100, :], start=True, stop=True)
                # p_t - 1 = 8*sigmoid(z) = 4*tanh(z/2) + 4
                nc.scalar.activation(t2[:], z2[0:2, :], AF.Tanh, scale=0.5)
                nc.vector.tensor_scalar(pm1[:], t2[:], 4.0, 4.0, ALU.mult, ALU.add)
                # floor (rounding-mode agnostic): c=int(x); c -= (c > x)
                nc.vector.tensor_copy(ci[:], pm1[:])
                nc.vector.tensor_copy(cf[:], ci[:])
                nc.vector.tensor_tensor(gt[:], cf[:], pm1[:], ALU.is_gt)
                nc.vector.tensor_tensor(st[:], cf[:], gt[:], ALU.subtract)
                # (clamp omitted: p_t-1 in (0,8) strictly, so floor in [0,7])
                # s16[m, b] = 10*st0[b] + st1[b] on all 128 partitions
                s16 = pmps.tile([128, 128], F32, tag="ps_z")
                nc.tensor.matmul(s16[:], comb2_sb, st[:], start=True, stop=True)
                # diagonal extract: base16[p, q] = s16[p, q*16 + p%16]
                nc.vector.tensor_tensor(s16m[:], s16[:], mask_sb, ALU.mult)
                nc.vector.reduce_sum(base16[:],
                                     s16m[:].rearrange("p (q i) -> p q i", i=16),
                                     axis=mybir.AxisListType.X)
                for j in range(WIN):
                    nc.vector.scalar_tensor_tensor(
                        idx16[:, j * 8:(j + 1) * 8], base16[:], float(G * j),
                        iota_sb, ALU.add, ALU.add)

            # ---- gather the 3x3 windows (3 gathers: one grid-row each) ----
            feat_gap = AP(feat.ap().tensor, 0, [[C, NROWS - 2], [1, WIN * C]])
            gathers = []
            for j in range(WIN):
                g = nc.gpsimd.dma_gather(
                    local_nat[:, j * WIN * C:(j + 1) * WIN * C]
                        .rearrange("p (o e) -> p o e", o=1),
                    feat_gap,
                    idx16[:, j * 8:(j + 1) * 8],
                    BS, BS, WIN * C,
                    elem_step=C,
                )
                add_dep_helper(g.ins, lib_inst.ins, reason="load_library before gather")
                gathers.append(g)

            with tc.tile_pool(name="pm2", bufs=2, space="PSUM") as pm2:
                # ---- w2h = (h @ W2)^T  [uo*128+m, b], one psum bank ----
                nc.vector.tensor_copy(hT16[:], hT32[:])
                psw = pm2.tile([128, 512], F32, tag="ps_w2h")
                for uo in range(KU):
                    for ui in range(KU):
                        nc.tensor.matmul(
                            psw[:, uo * 128:(uo + 1) * 128],
                            w2_sb[:, ui * U + uo * 128: ui * U + (uo + 1) * 128],
                            hT16[:, ui * 128:(ui + 1) * 128],
                            start=(ui == 0), stop=(ui == KU - 1))
                nc.vector.tensor_copy(w2h_sb[:], psw[:])
                # tanh(h)^T for the W3 matmul tail
                nc.scalar.activation(th16[:], hT32[:], AF.Tanh)

                # PE warmup during the gather/transpose window (keeps HAM at
                # full clock); results are never read.
                warm = pm2.tile([128, 512], F32, tag="ps_warm")
                for i in range(56):
                    nc.tensor.matmul(warm[:], w1_sb[:, 0:128],
                                     w1_sb[:, (i % 7) * 512:(i % 7) * 512 + 512],
                                     start=True, stop=True)

            # cast to bf16 + one multi-tile xbar transpose per grid row:
            # localT_j[c0, cc*384 + (l-3j)*128 + b] = local_bf[b, l*C+cc*128+c0]
            localTs = [localT0, localT1, localT2]
            for j in range(WIN):
                nc.vector.tensor_copy(local_bf[:, j * WIN * C:(j + 1) * WIN * C],
                                      local_nat[:, j * WIN * C:(j + 1) * WIN * C])
            for j in range(WIN):
                tr = nc.sync.dma_start(
                    localTs[j][:].rearrange("p (t b) -> p t b", b=128),
                    local_bf[:, j * WIN * C:(j + 1) * WIN * C],
                    transpose=True)
                for g in gathers:
                    add_dep_helper(tr.ins, g.ins,
                                   reason="xbar transpose after all gathers")

            # ---- scoreT = tanh(W1^T localT + w2h + b12)  [u, l*128+b] ----
            with tc.tile_pool(name="sps", bufs=2, space="PSUM") as sps:
                for uo in range(KU):
                    pss = [sps.tile([128, 384], F32, tag=f"ps_s{j}",
                                    name=f"pss{uo}_{j}")
                           for j in range(WIN)]
                    for k in range(KC):
                        for j in range(WIN):
                            nc.tensor.matmul(
                                pss[j][:],
                                w1_sb[:, k * U + uo * 128:k * U + (uo + 1) * 128],
                                localTs[j][:].rearrange(
                                    "p (li k b) -> p li k b", k=KC, b=128)[:, :, k, :],
                                start=(k == 0), stop=(k == KC - 1))
                    for j in range(WIN):
                        nc.vector.tensor_tensor(
                            stmp[:, j * 384:(j + 1) * 384]
                                .rearrange("p (l b) -> p l b", b=128),
                            pss[j][:].rearrange("p (l b) -> p l b", b=128),
                            w2h_sb[:, uo * 128:(uo + 1) * 128].unsqueeze(1)
                                .broadcast_to([128, WIN, 128]),
                            ALU.add)
                    nc.scalar.activation(scoreT[:, uo * RPW:(uo + 1) * RPW], stmp[:],
                                         AF.Tanh, bias=b12_sb[:, uo:uo + 1])

            # ---- logits -> softmax -> attn ----
            with tc.tile_pool(name="lgps", bufs=1, space="PSUM") as lgps:
                lg = lgps.tile([128, NL], F32, tag="ps_lg")
                for l in range(NL):
                    for uo in range(KU):
                        nc.tensor.matmul(
                            lg[:, l:l + 1],
                            scoreT[:, uo * RPW + l * 128:uo * RPW + (l + 1) * 128],
                            v1_sb[:, uo:uo + 1],
                            start=(uo == 0), stop=(uo == KU - 1))
                nc.vector.tensor_reduce(negmax[:], lg[:], axis=mybir.AxisListType.X,
                                        op=ALU.max, negate=True)
                nc.scalar.activation(e_sb[:], lg[:], AF.Exp, bias=negmax[:])
            nc.vector.reduce_sum(esum[:], e_sb[:], axis=mybir.AxisListType.X)
            nc.vector.reciprocal(rsum[:], esum[:])
            nc.vector.scalar_tensor_tensor(attn_f[:], e_sb[:], rsum[:], gauss_sb,
                                           ALU.mult, ALU.mult)
            nc.sync.dma_start(attn[:], attn_f[:])

            # ---- ctx^T via diag matmuls, then tanh -> tct16 ----
            for l in range(NL):
                nc.vector.tensor_scalar_mul(diag[:, l * 128:(l + 1) * 128], eye_sb,
                                            attn_f[:, l:l + 1])
            with tc.tile_pool(name="cps", bufs=4, space="PSUM") as cps, \
                 tc.tile_pool(name="ops", bufs=1, space="PSUM") as ops:
                for cc in range(KC):
                    pc = cps.tile([128, 128], F32, tag="ps_c", name=f"pc{cc}")
                    for l in range(NL):
                        nc.tensor.matmul(pc[:],
                                         local_bf[:, l * C + cc * 128:l * C + (cc + 1) * 128],
                                         diag[:, l * 128:(l + 1) * 128],
                                         start=(l == 0), stop=(l == NL - 1))
                    nc.scalar.activation(tct16[:, cc * 128:(cc + 1) * 128], pc[:], AF.Tanh)

                # ---- out = tanh([ctx, h]) @ W3 + W3_b ----
                po = ops.tile([128, U], F32, tag="ps_o")
                for kk in range(KT):
                    lhsT = (tct16[:, kk * 128:(kk + 1) * 128] if kk < KC
                            else th16[:, (kk - KC) * 128:(kk - KC + 1) * 128])
                    nc.tensor.matmul(po[:], lhsT, w3_sb[:, kk * U:(kk + 1) * U],
                                     start=(kk == 0), stop=(kk == KT - 1))
                nc.vector.tensor_tensor(out_sb[:], po[:], w3b_sb, ALU.add)
            nc.sync.dma_start(out[:], out_sb[:])

    nc.compile()
    return nc


_NC_CACHE = None


def _get_nc():
    global _NC_CACHE
    if _NC_CACHE is None:
        _NC_CACHE = _build_nc()
    return _NC_CACHE


def _chunked(w, k):
    """[k*128, n] -> [128, k*n] with chunk-major columns."""
    n = w.shape[1]
    return np.ascontiguousarray(
        w.reshape(k, 128, n).transpose(1, 0, 2).reshape(128, k * n))


def make_host_inputs(features, hidden, W1_w, W1_b, W2_w, W2_b, V1_w, V1_b,
                     W3_w, W3_b, Wa, Wb):
    """Build the 8 per-core input maps."""
    bf = ml_dtypes.bfloat16
    f = np.float32

    jj, kk = np.meshgrid(np.arange(WIN), np.arange(WIN), indexing="ij")
    d2 = (jj - WIN / 2.0) ** 2 + (kk - WIN / 2.0) ** 2
    gauss_row = np.exp(-d2 / (0.5 * D * D)).reshape(NL).astype(f)

    p = np.arange(128)
    q = np.arange(8)

    w3cat = np.zeros((128, KT * U + 128 + KU), bf)
    w3cat[:, 0:KT * U] = _chunked(np.asarray(W3_w, f), KT).astype(bf)
    w3cat[:, KT * U:KT * U + 128] = np.eye(128, dtype=f).astype(bf)
    w3cat[:, KT * U + 128:] = _chunked(np.asarray(V1_w, f), KU).astype(bf)

    fcrit = np.zeros((128, FC_COLS), f)
    fcrit[:, F_ID:F_ID + 128] = np.eye(128, dtype=f)
    fcrit[:, F_WA:F_WA + KU * 100] = _chunked(np.asarray(Wa, f), KU)
    fcrit[:, F_MASK:F_MASK + 128] = (
        np.arange(128)[None, :] % 16 == p[:, None] % 16)
    fcrit[0, F_COMB:F_COMB + 128] = float(G)
    fcrit[1, F_COMB:F_COMB + 128] = 1.0

    frest = np.zeros((128, FR_COLS), f)
    frest[:, F_W3B:F_GAUSS] = np.broadcast_to(np.asarray(W3_b, f), (128, U))
    frest[:, F_GAUSS:F_IOTA] = np.broadcast_to(gauss_row, (128, NL))
    frest[:, F_IOTA:F_B12] = L * (q[None, :] * 16 + (p[:, None] % 16))
    frest[:, F_B12:F_B12 + KU] = _chunked(
        (np.asarray(W1_b, f) + np.asarray(W2_b, f)).reshape(U, 1), KU)

    si = np.arange(24)[None, :] * 16 + (np.arange(128)[:, None] % 16)
    shared = {
        "sgi": si.astype(np.int16),
        "fcrit": fcrit,
        "frest": frest,
        "w1cat": _chunked(np.asarray(W1_w, f), KC).astype(bf),
        "w2cat": _chunked(np.asarray(W2_w, f), KU).astype(bf),
        "w3cat": w3cat,
        "wbd": np.ascontiguousarray(Wb, f),
    }
    features = np.asarray(features, f)
    hidden = np.asarray(hidden, f)
    in_maps = []
    for c in range(NC_CORES):
        sl = slice(c * BS, (c + 1) * BS)
        m = dict(shared)
        m["feat"] = np.ascontiguousarray(features[sl]).reshape(NROWS, C)
        m["hid"] = np.ascontiguousarray(hidden[sl])
        in_maps.append(m)
    return in_maps


def kernel(features, hidden, W1_w, W1_b, W2_w, W2_b, V1_w, V1_b,
           W3_w, W3_b, Wa, Wb, _run_kwargs=None):
    nc = _get_nc()
    in_maps = make_host_inputs(features, hidden, W1_w, W1_b, W2_w, W2_b,
                               V1_w, V1_b, W3_w, W3_b, Wa, Wb)
    res = run_bass_kernel_spmd(nc, in_maps, core_ids=list(range(NC_CORES)),
                               **(_run_kwargs or {}))
    out = np.concatenate([r["out"] for r in res.results], axis=0)
    attn = np.concatenate([r["attn"] for r in res.results], axis=0)
    kernel.last_results = res
    return out, attn.reshape(B, NL, 1)
